# revision 3
# baseline (speedup 1.0000x reference)
"""Multi-head causal attention block (B=2, S=2048, F=1024, H=16, D=64)
on 8 TRN2 NeuronCores.

Sharding: core = 4*b + g  (b = batch 0..1, g = head-group 0..3, 4 heads each).
Each core computes, for its batch and its 4 heads:
  qkv projection (columns of w_attn for its heads), causal attention,
  and the partial output projection (rows of w_proj for its heads).
Host sums the 4 per-group partials per batch and adds the bias constant
(b_proj + b_attn_v @ w_proj, which is token-independent).

On-chip layout ("orientation B" — scores transposed, no P-transposes):
  xT    [f=128x8, s]   built from x via PE transposes
  qkT   [dim=128x4, s] = (wqk^T x^T); chunks: [q_h0|q_h1], [q_h2|q_h3],
                         [k_h0|k_h1], [k_h2|k_h3]  (64 partitions per head)
  v     [s=128x16, d=64x4 (+ones col per head)]
  per head h, per sq-chunk c (512 wide):
    for sk-tile t <= diag: sT = matmul(lhsT=kT_h[:, t], rhs=qT_h[:, c])
      -> [sk=128, sq=512] PSUM (+causal mask on diag tiles),
      exp on ACT -> SBUF f32r,
      zT'[65, 512] += v_ones_h[:,t].T @ expP   (PSUM accumulate; row 64 = denom)
    normalize: z = zT'[:64] * broadcast(1/zT'[64])
  out partial [s=128, f] = zTm.T @ wp  (2 c-chunks accumulate)

All matmuls run in float32r (full-rate fp32; ~2^-14 operand rounding).
"""

import numpy as np

import concourse.mybir as mybir
import concourse.tile as tile
from concourse import bacc
from concourse.bass_utils import run_bass_kernel_spmd
from concourse.masks import make_identity

B, S, F, H, D = 2, 2048, 1024, 16, 64
P = 128
NCORES = 8
HPC = 4  # heads per core
GD = HPC * D  # 256 dims per head group
ST = S // P  # 16 sequence tiles
FC = F // P  # 8 feature chunks
SQC = 4  # sq chunks of 512
CW = 512  # chunk width
NEG = -1.0e9

f32 = mybir.dt.float32
f32r = mybir.dt.float32r

_cached_nc = None


def build_nc():
    nc = bacc.Bacc("TRN2", target_bir_lowering=False, debug=False,
                   num_devices=NCORES)
    x = nc.dram_tensor("x", [S, F], f32, kind="ExternalInput")
    wqk = nc.dram_tensor("wqk", [F, 2 * GD], f32, kind="ExternalInput")
    wv = nc.dram_tensor("wv", [F, GD], f32, kind="ExternalInput")
    wp = nc.dram_tensor("wp", [GD, F], f32, kind="ExternalInput")
    bqk = nc.dram_tensor("bqk", [P, 4], f32, kind="ExternalInput")
    out = nc.dram_tensor("out", [S, F], f32, kind="ExternalOutput")

    with tile.TileContext(nc) as tc:
        with (
            tc.tile_pool(name="consts", bufs=1) as consts,
            tc.tile_pool(name="stage", bufs=1) as stage,
            tc.tile_pool(name="work", bufs=3) as work,
            tc.tile_pool(name="ps_s", bufs=3, space="PSUM") as ps_s,
            tc.tile_pool(name="ps_z", bufs=3, space="PSUM") as ps_z,
            tc.tile_pool(name="ps_m", bufs=2, space="PSUM") as ps_m,
        ):
            # ---- constants ----
            ident = consts.tile([P, P], f32)
            make_identity(nc, ident[:])
            ones = consts.tile([P, 1], f32)
            nc.vector.memset(ones[:], 1.0)
            # causal additive mask: mbig[i, y] = 0 if y >= i + 384 else NEG
            mbig = consts.tile([P, 896], f32)
            nc.gpsimd.memset(mbig[:], 0.0)
            nc.gpsimd.affine_select(
                out=mbig[:], in_=mbig[:],
                compare_op=mybir.AluOpType.is_ge, fill=NEG,
                base=-384, pattern=[[1, 896]], channel_multiplier=-1,
            )
            bqk_sb = consts.tile([P, 4], f32)
            nc.sync.dma_start(bqk_sb[:], bqk[:])

            # ---- stream x tiles and build xT (f on partitions) ----
            # xT shares its slot with zTm (disjoint lifetimes) via tag "bigB"
            xT = stage.tile([P, FC, S], f32r, tag="bigB", name="xT")
            for t in range(ST):
                xtile = work.tile([P, F], f32, tag="xtile")
                nc.sync.dma_start(xtile[:], x[t * P:(t + 1) * P, :])
                for fc in range(FC):
                    tps = ps_m.tile([P, P], f32, tag="mps", name="tps")
                    nc.tensor.transpose(
                        tps[:], xtile[:, fc * P:(fc + 1) * P], ident[:]
                    )
                    nc.vector.tensor_copy(
                        xT[:, fc, t * P:(t + 1) * P], tps[:]
                    )

            # ---- round weights to f32r ----
            wqk_f = stage.tile([P, FC, 2 * GD], f32, tag="wtmp", name="wqk_f")
            nc.sync.dma_start(
                wqk_f[:], wqk[:].rearrange("(c p) n -> p c n", p=P)
            )
            wqk_r = stage.tile([P, FC, 2 * GD], f32r, tag="wr1", name="wqk_r")
            nc.vector.tensor_copy(wqk_r[:], wqk_f[:])

            wv_f = stage.tile([P, FC, GD], f32, tag="wtmp", name="wv_f")
            nc.sync.dma_start(
                wv_f[:], wv[:].rearrange("(c p) n -> p c n", p=P)
            )
            wv_r = stage.tile([P, FC, GD], f32r, tag="wvr", name="wv_r")
            nc.vector.tensor_copy(wv_r[:], wv_f[:])

            # ---- qkT projection: 4 out-chunks of 128 dims ----
            qkT = stage.tile([P, 4, S], f32r, tag="qkT", name="qkT")
            for oc in range(4):
                for c in range(SQC):
                    pp = ps_m.tile([P, CW], f32, tag="mps", name="projps")
                    for fc in range(FC):
                        nc.tensor.matmul(
                            pp[:],
                            wqk_r[:, fc, oc * P:(oc + 1) * P],
                            xT[:, fc, c * CW:(c + 1) * CW],
                            start=(fc == 0), stop=(fc == FC - 1),
                        )
                    nc.scalar.activation(
                        qkT[:, oc, c * CW:(c + 1) * CW], pp[:],
                        mybir.ActivationFunctionType.Identity,
                        bias=bqk_sb[:, oc:oc + 1],
                    )

            # ---- v projection: [head, sk tile, head dims + ones col] ----
            vt = stage.tile([P, HPC, ST, D + 1], f32r, tag="vt", name="vt")
            for h in range(HPC):
                nc.vector.tensor_copy(
                    vt[:, h, :, D:D + 1],
                    ones[:, None, :].to_broadcast((P, ST, 1)),
                )
            for t in range(ST):
                pp = ps_m.tile([P, GD], f32, tag="mps", name="projps")
                for fc in range(FC):
                    nc.tensor.matmul(
                        pp[:],
                        xT[:, fc, t * P:(t + 1) * P],
                        wv_r[:, fc, :],
                        start=(fc == 0), stop=(fc == FC - 1),
                    )
                for h in range(HPC):
                    nc.vector.tensor_copy(
                        vt[:, h, t, :D], pp[:, h * D:(h + 1) * D]
                    )

            # ---- attention, head-pair interleaved for PE row packing ----
            # zTm reuses xT's slot (xT dead after v proj)
            zTm = stage.tile([P, 2, S], f32r, tag="bigB", name="zTm")
            for c in range(SQC):
                for hp in range(2):  # head pair (2*hp, 2*hp+1)
                    zps = [
                        ps_z.tile([P, CW], f32, tag="zps", name=f"zps{i}")
                        for i in range(2)
                    ]
                    nt = 4 * c + 4
                    for t in range(nt):
                        for i in range(2):
                            h = 2 * hp + i
                            lo = (h % 2) * D
                            sps = ps_s.tile([P, CW], f32, tag="sps",
                                            name="sps")
                            nc.tensor.matmul(
                                sps[:],
                                qkT[lo:lo + D, 2 + h // 2,
                                    t * P:(t + 1) * P],
                                qkT[lo:lo + D, h // 2,
                                    c * CW:(c + 1) * CW],
                                start=True, stop=True,
                                skip_group_check=True,
                            )
                            if t >= 4 * c:
                                w = (t - 4 * c) * P + P
                                nc.vector.tensor_add(
                                    sps[:, 0:w], sps[:, 0:w],
                                    mbig[:, 512 - w:512],
                                )
                            ep = work.tile([P, CW], f32r, tag="ep")
                            nc.scalar.activation(
                                ep[:], sps[:],
                                mybir.ActivationFunctionType.Exp,
                            )
                            nc.tensor.matmul(
                                zps[i][:D + 1],
                                vt[:, h, t, :],
                                ep[:],
                                start=(t == 0), stop=(t == nt - 1),
                                skip_group_check=True,
                            )
                    for i in range(2):
                        h = 2 * hp + i
                        rec = work.tile([1, CW], f32, tag="rec")
                        nc.vector.reciprocal(rec[:], zps[i][D:D + 1, :])
                        recb = work.tile([D, CW], f32, tag="recb")
                        nc.gpsimd.partition_broadcast(recb[:], rec[:])
                        lo = (h % 2) * D
                        nc.vector.tensor_mul(
                            zTm[lo:lo + D, h // 2, c * CW:(c + 1) * CW],
                            zps[i][:D], recb[:],
                        )

            # ---- output projection ----
            wp_f = stage.tile([P, 2, F], f32, tag="wtmp", name="wp_f")
            nc.sync.dma_start(wp_f[:], wp[:].rearrange("(c p) n -> p c n", p=P))
            wp_r = stage.tile([P, 2, F], f32r, tag="wr1", name="wp_r")
            nc.vector.tensor_copy(wp_r[:], wp_f[:])

            for t in range(ST):
                osb = work.tile([P, F], f32, tag="osb")
                for n in range(2):
                    pp = ps_m.tile([P, CW], f32, tag="mps", name="projps")
                    for cc in range(2):
                        nc.tensor.matmul(
                            pp[:],
                            zTm[:, cc, t * P:(t + 1) * P],
                            wp_r[:, cc, n * CW:(n + 1) * CW],
                            start=(cc == 0), stop=(cc == 1),
                        )
                    nc.scalar.copy(osb[:, n * CW:(n + 1) * CW], pp[:])
                nc.sync.dma_start(out[t * P:(t + 1) * P, :], osb[:])
    nc.compile()
    return nc


def make_in_maps(x, w_attn, b_attn, w_proj):
    x = np.ascontiguousarray(np.asarray(x, dtype=np.float32))
    w_attn = np.asarray(w_attn, dtype=np.float32)
    b_attn = np.asarray(b_attn, dtype=np.float32)
    w_proj = np.ascontiguousarray(np.asarray(w_proj, dtype=np.float32))
    scale = np.float32(1.0 / np.sqrt(D))
    in_maps = []
    for core in range(NCORES):
        b, g = divmod(core, 4)
        sl = slice(g * GD, (g + 1) * GD)
        wq = w_attn[:, sl] * scale
        wk = w_attn[:, F + g * GD:F + (g + 1) * GD]
        wqkm = np.ascontiguousarray(
            np.concatenate([wq, wk], axis=1), dtype=np.float32
        )
        wvm = np.ascontiguousarray(
            w_attn[:, 2 * F + g * GD:2 * F + (g + 1) * GD]
        )
        wpg = np.ascontiguousarray(w_proj[sl, :])
        bq = b_attn[sl] * scale
        bk = b_attn[F + g * GD:F + (g + 1) * GD]
        bqkm = np.ascontiguousarray(
            np.concatenate([bq, bk]).reshape(4, P).T, dtype=np.float32
        )
        in_maps.append(
            {"x": np.ascontiguousarray(x[b]), "wqk": wqkm, "wv": wvm,
             "wp": wpg, "bqk": bqkm}
        )
    return in_maps


def assemble(results, b_attn, b_proj, w_proj):
    b_attn = np.asarray(b_attn, dtype=np.float64)
    b_proj = np.asarray(b_proj, dtype=np.float64)
    w_proj = np.asarray(w_proj, dtype=np.float64)
    const = b_attn[2 * F:] @ w_proj + b_proj  # token-independent v-bias term
    full = np.empty((B, S, F), dtype=np.float32)
    for b in range(B):
        acc = results[4 * b]["out"].astype(np.float64)
        for g in range(1, 4):
            acc = acc + results[4 * b + g]["out"]
        full[b] = (acc + const).astype(np.float32)
    return full


def kernel(x, w_attn, b_attn, w_proj, b_proj):
    global _cached_nc
    if _cached_nc is None:
        _cached_nc = build_nc()
    in_maps = make_in_maps(x, w_attn, b_attn, w_proj)
    res = run_bass_kernel_spmd(
        _cached_nc, in_maps, core_ids=list(range(NCORES))
    )
    return assemble(res.results, b_attn, b_proj, w_proj)


# revision 7
# speedup vs baseline: 1.0588x; 1.0588x over previous
"""Multi-head causal attention block (B=2, S=2048, F=1024, H=16, D=64)
on 8 TRN2 NeuronCores.

Sharding: core = 4*b + g  (b = batch 0..1, g = head-group 0..3, 4 heads each).
Each core computes, for its batch and its 4 heads:
  qkv projection (columns of w_attn for its heads), causal attention,
  and the partial output projection (rows of w_proj for its heads).
Host sums the 4 per-group partials per batch and adds the bias constant
(b_proj + b_attn_v @ w_proj, which is token-independent).

On-chip dataflow ("orientation B" — scores transposed, no P-transposes of
the attention weights):
  xT   [f, s]   via PE transposes (4 per PSUM bank, batched copy-out)
  qkT  [dim, s] = wqk^T @ xT; chunks [q_h0|q_h1],[q_h2|q_h3],[k_h0|k_h1],[k_h2|k_h3]
  v    [s, d]   direct orientation, +ones column per head (denominator row)
  attention per head h, sq-chunk c (512 wide), sk tile t<=diag:
    sT = matmul(lhsT=kT_h[:,t], rhs=qT_h[:,chunk])  [sk=128, sq<=512] PSUM
    (pairs of t share one 2-bank PSUM tile; one exp op per pair)
    exp on ACT -> SBUF f32r; causal triangle zeroed by GPSIMD affine_select
    zT'[65, 512] += v_ones_h[:,t].T @ expP  (PSUM accumulate; row 64 = denom)
    normalize: z = zT'[:64] * bcast(approx_recip(zT'[64]))
  out partial [s, f] = zTm.T @ wp

Diag tiles compute only the valid sq range (width 512-off), so there is no
wasted score/exp/AV work beyond the masked 128x128 triangle.

Everything is emitted chunk-pipelined (transpose(c) -> proj(c) ->
attention(c) -> outproj(c)) so the Tile scheduler can overlap phases and
keep the PE dense (HAM stays un-throttled).

All matmuls run in float32r (full-rate fp32; ~2^-14 operand rounding).
"""

import numpy as np

import concourse.mybir as mybir
import concourse.tile as tile
from concourse import bacc
from concourse.bass_utils import run_bass_kernel_spmd
from concourse.masks import make_identity

B, S, F, H, D = 2, 2048, 1024, 16, 64
P = 128
NCORES = 8
HPC = 4  # heads per core
GD = HPC * D  # 256 dims per head group
ST = S // P  # 16 sequence tiles
FC = F // P  # 8 feature chunks
SQC = 4  # sq chunks of 512
CW = 512  # chunk width
NEG = -1.0e9

f32 = mybir.dt.float32
f32r = mybir.dt.float32r

_cached_nc = None


def build_nc():
    nc = bacc.Bacc("TRN2", target_bir_lowering=False, debug=False,
                   num_devices=NCORES)
    x = nc.dram_tensor("x", [S, F], f32, kind="ExternalInput")
    wqk = nc.dram_tensor("wqk", [F, 2 * GD], f32, kind="ExternalInput")
    wv = nc.dram_tensor("wv", [F, GD], f32, kind="ExternalInput")
    wp = nc.dram_tensor("wp", [GD, F], f32, kind="ExternalInput")
    bqk = nc.dram_tensor("bqk", [P, 4], f32, kind="ExternalInput")
    out = nc.dram_tensor("out", [S, F], f32, kind="ExternalOutput")

    with tile.TileContext(nc) as tc:
        with (
            tc.tile_pool(name="consts", bufs=1) as consts,
            tc.tile_pool(name="stage", bufs=1) as stage,
            tc.tile_pool(name="work", bufs=2) as work,
            tc.tile_pool(name="eps", bufs=2) as eps,
            tc.tile_pool(name="ps_s", bufs=2, space="PSUM") as ps_s,
            tc.tile_pool(name="ps_z", bufs=3, space="PSUM") as ps_z,
            tc.tile_pool(name="ps_m", bufs=1, space="PSUM") as ps_m,
        ):
            # ---- constants ----
            ident = consts.tile([P, P], f32)
            make_identity(nc, ident[:])
            ones = consts.tile([P, 1], f32)
            nc.vector.memset(ones[:], 1.0)
            bqk_sb = consts.tile([P, 4], f32)
            nc.sync.dma_start(bqk_sb[:], bqk[:])

            # ---- round weights to f32r (streamed through small tiles) ----
            wqk_r = stage.tile([P, FC, 2 * GD], f32r, tag="wqk_r", name="wqk_r")
            wv_r = stage.tile([P, FC, GD], f32r, tag="wv_r", name="wv_r")
            wp_r = stage.tile([P, 2, F], f32r, tag="wp_r", name="wp_r")
            for fc in range(FC):
                wt = work.tile([P, 2 * GD], f32, tag="wtmp", name="wt_qk")
                nc.sync.dma_start(wt[:], wqk[fc * P:(fc + 1) * P, :])
                nc.vector.tensor_copy(wqk_r[:, fc, :], wt[:])
            for fc in range(FC):
                wt = work.tile([P, GD], f32, tag="wtmp", name="wt_v")
                nc.sync.dma_start(wt[:], wv[fc * P:(fc + 1) * P, :])
                nc.vector.tensor_copy(wv_r[:, fc, :], wt[:])
            for cc in range(2):
                wt = work.tile([P, F], f32, tag="wtmp", name="wt_p")
                nc.sync.dma_start(wt[:], wp[cc * P:(cc + 1) * P, :])
                nc.vector.tensor_copy(wp_r[:, cc, :], wt[:])

            # ---- persistent activations ----
            xT = stage.tile([P, FC, S], f32r, tag="xT", name="xT")
            qkT = stage.tile([P, 4, S], f32r, tag="qkT", name="qkT")
            vt = stage.tile([P, HPC, ST, D + 1], f32r, tag="vt", name="vt")
            zTm = stage.tile([P, 2, S], f32r, tag="zTm", name="zTm")
            for h in range(HPC):
                nc.vector.tensor_copy(
                    vt[:, h, :, D:D + 1],
                    ones[:, None, :].to_broadcast((P, ST, 1)),
                )

            def transposes(c):
                for tt in range(4):
                    t = 4 * c + tt
                    xt_ = work.tile([P, F], f32, tag="xtile", name="xtile")
                    nc.sync.dma_start(xt_[:], x[t * P:(t + 1) * P, :])
                    for half in range(2):
                        pp = ps_m.tile([P, CW], f32, tag="mps", name="tps")
                        for q in range(4):
                            fc = half * 4 + q
                            nc.tensor.transpose(
                                pp[:, q * P:(q + 1) * P],
                                xt_[:, fc * P:(fc + 1) * P],
                                ident[:],
                            )
                        nc.vector.tensor_copy(
                            xT[:, half * 4:half * 4 + 4,
                               t * P:(t + 1) * P],
                            pp[:].rearrange("p (f q) -> p f q", f=4),
                        )

            def qkproj(c):
                for oc in range(4):
                    pp = ps_m.tile([P, CW], f32, tag="mps", name="qkps")
                    for fc in range(FC):
                        nc.tensor.matmul(
                            pp[:],
                            wqk_r[:, fc, oc * P:(oc + 1) * P],
                            xT[:, fc, c * CW:(c + 1) * CW],
                            start=(fc == 0), stop=(fc == FC - 1),
                        )
                    nc.vector.tensor_tensor(
                        qkT[:, oc, c * CW:(c + 1) * CW], pp[:],
                        bqk_sb[:, oc:oc + 1].to_broadcast((P, CW)),
                        mybir.AluOpType.add,
                    )

            def vproj(c):
                for tt in range(4):
                    t = 4 * c + tt
                    pp = ps_m.tile([P, GD], f32, tag="mps", name="vps")
                    for fc in range(FC):
                        nc.tensor.matmul(
                            pp[:],
                            xT[:, fc, t * P:(t + 1) * P],
                            wv_r[:, fc, :],
                            start=(fc == 0), stop=(fc == FC - 1),
                        )
                    nc.vector.tensor_copy(
                        vt[:, :, t, :D],
                        pp[:].rearrange("p (h d) -> p h d", h=HPC),
                    )

            def av(zp, h, t, ep_ap, col0, ncols, start, stop):
                nc.tensor.matmul(
                    zp[:D + 1, col0:col0 + ncols],
                    vt[:, h, t, :],
                    ep_ap,
                    start=start, stop=stop,
                    skip_group_check=True,
                )

            def scores(sp_ap, h, t, c, q0, qw):
                lo = (h % 2) * D
                nc.tensor.matmul(
                    sp_ap,
                    qkT[lo:lo + D, 2 + h // 2, t * P:(t + 1) * P],
                    qkT[lo:lo + D, h // 2, c * CW + q0:c * CW + q0 + qw],
                    start=True, stop=True,
                    skip_group_check=True,
                )

            def diag_mask(ep_ap):
                # zero the strictly-lower triangle: valid iff jloc >= i
                nc.gpsimd.affine_select(
                    out=ep_ap, in_=ep_ap,
                    compare_op=mybir.AluOpType.is_ge,
                    fill=0.0, base=0,
                    pattern=[[1, P]], channel_multiplier=-1,
                )

            def attention(c):
                for hp in range(2):
                    heads = (2 * hp, 2 * hp + 1)
                    zps = [
                        ps_z.tile([P, CW], f32, tag="zps", name=f"zps{i}")
                        for i in range(2)
                    ]
                    # off-diagonal pairs (full width)
                    for pair in range(2 * c):
                        t0, t1 = 2 * pair, 2 * pair + 1
                        for i, h in enumerate(heads):
                            sp = ps_s.tile([P, 2 * CW], f32, tag="sps",
                                           name="sps")
                            scores(sp[:, 0:CW], h, t0, c, 0, CW)
                            scores(sp[:, CW:2 * CW], h, t1, c, 0, CW)
                            ep = eps.tile([P, 2 * CW], f32r, tag="ep",
                                          name="ep")
                            nc.scalar.activation(
                                ep[:], sp[:],
                                mybir.ActivationFunctionType.Exp,
                            )
                            first = (t0 == 0)
                            av(zps[i], h, t0, ep[:, 0:CW], 0, CW,
                               first, False)
                            av(zps[i], h, t1, ep[:, CW:2 * CW], 0, CW,
                               False, False)
                    # diagonal pairs: widths (512, 384) and (256, 128)
                    for dp in range(2):
                        ta, tb = 4 * c + 2 * dp, 4 * c + 2 * dp + 1
                        offa, offb = 2 * dp * P, (2 * dp + 1) * P
                        wa, wb = CW - offa, CW - offb
                        for i, h in enumerate(heads):
                            sp = ps_s.tile([P, 2 * CW], f32, tag="sps",
                                           name="sps")
                            scores(sp[:, 0:wa], h, ta, c, offa, wa)
                            scores(sp[:, wa:wa + wb], h, tb, c, offb, wb)
                            ep = eps.tile([P, 2 * CW], f32r, tag="ep",
                                          name="ep")
                            nc.scalar.activation(
                                ep[:, 0:wa + wb], sp[:, 0:wa + wb],
                                mybir.ActivationFunctionType.Exp,
                            )
                            diag_mask(ep[:, 0:P])
                            diag_mask(ep[:, wa:wa + P])
                            first = (c == 0 and dp == 0)
                            av(zps[i], h, ta, ep[:, 0:wa], offa, wa,
                               first, False)
                            av(zps[i], h, tb, ep[:, wa:wa + wb], offb, wb,
                               False, (dp == 1))
                    # normalize
                    for i, h in enumerate(heads):
                        den = work.tile([1, CW], f32, tag="den", name="den")
                        nc.vector.tensor_copy(den[:], zps[i][D:D + 1, :])
                        rec = work.tile([1, CW], f32, tag="rec", name="rec")
                        nc.vector.reciprocal_approx_fast(rec[:], den[:])
                        recb = work.tile([D, CW], f32, tag="recb",
                                         name="recb")
                        nc.gpsimd.partition_broadcast(recb[:], rec[:])
                        lo = (h % 2) * D
                        nc.vector.tensor_mul(
                            zTm[lo:lo + D, h // 2, c * CW:(c + 1) * CW],
                            zps[i][:D], recb[:],
                        )

            def outproj(c):
                for tt in range(4):
                    t = 4 * c + tt
                    osb = work.tile([P, F], f32, tag="osb", name="osb")
                    for n in range(2):
                        pp = ps_m.tile([P, CW], f32, tag="mps", name="ops")
                        for cc in range(2):
                            nc.tensor.matmul(
                                pp[:],
                                zTm[:, cc, t * P:(t + 1) * P],
                                wp_r[:, cc, n * CW:(n + 1) * CW],
                                start=(cc == 0), stop=(cc == 1),
                            )
                        nc.vector.tensor_copy(osb[:, n * CW:(n + 1) * CW],
                                              pp[:])
                    nc.sync.dma_start(out[t * P:(t + 1) * P, :], osb[:])

            for c in range(SQC):
                transposes(c)
                qkproj(c)
                vproj(c)
                attention(c)
                outproj(c)
    nc.compile()
    return nc


def make_in_maps(x, w_attn, b_attn, w_proj):
    x = np.ascontiguousarray(np.asarray(x, dtype=np.float32))
    w_attn = np.asarray(w_attn, dtype=np.float32)
    b_attn = np.asarray(b_attn, dtype=np.float32)
    w_proj = np.ascontiguousarray(np.asarray(w_proj, dtype=np.float32))
    scale = np.float32(1.0 / np.sqrt(D))
    in_maps = []
    for core in range(NCORES):
        b, g = divmod(core, 4)
        sl = slice(g * GD, (g + 1) * GD)
        wq = w_attn[:, sl] * scale
        wk = w_attn[:, F + g * GD:F + (g + 1) * GD]
        wqkm = np.ascontiguousarray(
            np.concatenate([wq, wk], axis=1), dtype=np.float32
        )
        wvm = np.ascontiguousarray(
            w_attn[:, 2 * F + g * GD:2 * F + (g + 1) * GD]
        )
        wpg = np.ascontiguousarray(w_proj[sl, :])
        bq = b_attn[sl] * scale
        bk = b_attn[F + g * GD:F + (g + 1) * GD]
        bqkm = np.ascontiguousarray(
            np.concatenate([bq, bk]).reshape(4, P).T, dtype=np.float32
        )
        in_maps.append(
            {"x": np.ascontiguousarray(x[b]), "wqk": wqkm, "wv": wvm,
             "wp": wpg, "bqk": bqkm}
        )
    return in_maps


def assemble(results, b_attn, b_proj, w_proj):
    b_attn = np.asarray(b_attn, dtype=np.float64)
    b_proj = np.asarray(b_proj, dtype=np.float64)
    w_proj = np.asarray(w_proj, dtype=np.float64)
    const = b_attn[2 * F:] @ w_proj + b_proj  # token-independent v-bias term
    full = np.empty((B, S, F), dtype=np.float32)
    for b in range(B):
        acc = results[4 * b]["out"].astype(np.float64)
        for g in range(1, 4):
            acc = acc + results[4 * b + g]["out"]
        full[b] = (acc + const).astype(np.float32)
    return full


def kernel(x, w_attn, b_attn, w_proj, b_proj):
    global _cached_nc
    if _cached_nc is None:
        _cached_nc = build_nc()
    in_maps = make_in_maps(x, w_attn, b_attn, w_proj)
    res = run_bass_kernel_spmd(
        _cached_nc, in_maps, core_ids=list(range(NCORES))
    )
    return assemble(res.results, b_attn, b_proj, w_proj)


# revision 10
# speedup vs baseline: 1.4239x; 1.3449x over previous
"""Multi-head causal attention block (B=2, S=2048, F=1024, H=16, D=64)
on 8 TRN2 NeuronCores.

Sharding: core = 4*b + g  (b = batch 0..1, g = head-group 0..3, 4 heads each).
Each core computes, for its batch and its 4 heads:
  qkv projection (columns of w_attn for its heads), causal attention,
  and the partial output projection (rows of w_proj for its heads).
Host sums the 4 per-group partials per batch and adds the bias constant
(b_proj + b_attn_v @ w_proj, which is token-independent).

On-chip dataflow ("orientation B" — scores transposed, no P-transposes of
the attention weights):
  xT   [f, s]   via PE transposes (4 per PSUM bank, batched copy-out)
  qkT  [dim, s] = wqk^T @ xT; chunks [q_h0|q_h1],[q_h2|q_h3],[k_h0|k_h1],[k_h2|k_h3]
  v    [s, d]   direct orientation, +ones column per head (denominator row)
  attention per head h, sq-chunk c (512 wide), sk tile t<=diag:
    sT = matmul(lhsT=kT_h[:,t], rhs=qT_h[:,chunk])  [sk=128, sq<=512] PSUM
    (pairs of t share one 2-bank PSUM tile; one exp op per pair)
    exp on ACT -> SBUF f32r; causal triangle zeroed by GPSIMD affine_select
    zT'[65, 512] += v_ones_h[:,t].T @ expP  (PSUM accumulate; row 64 = denom)
    normalize: z = zT'[:64] * bcast(approx_recip(zT'[64]))
  out partial [s, f] = zTm.T @ wp

Diag tiles compute only the valid sq range (width 512-off), so there is no
wasted score/exp/AV work beyond the masked 128x128 triangle.

Everything is emitted chunk-pipelined (transpose(c) -> proj(c) ->
attention(c) -> outproj(c)) so the Tile scheduler can overlap phases and
keep the PE dense (HAM stays un-throttled).

All matmuls run in float32r (full-rate fp32; ~2^-14 operand rounding).
"""

import numpy as np

import concourse.mybir as mybir
import concourse.tile as tile
from concourse import bacc
from concourse.bass_utils import run_bass_kernel_spmd
from concourse.masks import make_identity

B, S, F, H, D = 2, 2048, 1024, 16, 64
P = 128
NCORES = 8
HPC = 4  # heads per core
GD = HPC * D  # 256 dims per head group
ST = S // P  # 16 sequence tiles
FC = F // P  # 8 feature chunks
SQC = 4  # sq chunks of 512
CW = 512  # chunk width
NEG = -1.0e9

f32 = mybir.dt.float32
f32r = mybir.dt.float32r

_cached_nc = None


def build_nc():
    nc = bacc.Bacc("TRN2", target_bir_lowering=False, debug=False,
                   num_devices=NCORES)
    x = nc.dram_tensor("x", [S, F], f32, kind="ExternalInput")
    wqk = nc.dram_tensor("wqk", [F, 2 * GD], f32, kind="ExternalInput")
    wv = nc.dram_tensor("wv", [F, GD], f32, kind="ExternalInput")
    wp = nc.dram_tensor("wp", [GD, F], f32, kind="ExternalInput")
    bqk = nc.dram_tensor("bqk", [P, 4], f32, kind="ExternalInput")
    out = nc.dram_tensor("out", [S, F], f32, kind="ExternalOutput")

    with tile.TileContext(nc) as tc:
        with (
            tc.tile_pool(name="consts", bufs=1) as consts,
            tc.tile_pool(name="stage", bufs=1) as stage,
            tc.tile_pool(name="work", bufs=2) as work,
            tc.tile_pool(name="eps", bufs=2) as eps,
            tc.tile_pool(name="ps_s", bufs=2, space="PSUM") as ps_s,
            tc.tile_pool(name="ps_z", bufs=2, space="PSUM") as ps_z,
            tc.tile_pool(name="ps_m", bufs=2, space="PSUM") as ps_m,
        ):
            # ---- constants ----
            ident = consts.tile([P, P], f32)
            make_identity(nc, ident[:])
            ones = consts.tile([P, 1], f32)
            nc.vector.memset(ones[:], 1.0)
            bqk_sb = consts.tile([P, 4], f32)
            nc.sync.dma_start(bqk_sb[:], bqk[:])
            # additive causal triangle: keep iff jloc >= i, else -1e9
            mask128 = consts.tile([P, P], f32)
            nc.gpsimd.memset(mask128[:], 0.0)
            nc.gpsimd.affine_select(
                out=mask128[:], in_=mask128[:],
                compare_op=mybir.AluOpType.is_ge,
                fill=NEG, base=0,
                pattern=[[1, P]], channel_multiplier=-1,
            )

            # ---- round weights to f32r (streamed through small tiles) ----
            wqk_r = stage.tile([P, FC, 2 * GD], f32r, tag="wqk_r", name="wqk_r")
            wv_r = stage.tile([P, FC, GD], f32r, tag="wv_r", name="wv_r")
            wp_r = stage.tile([P, 2, F], f32r, tag="wp_r", name="wp_r")
            for fc in range(FC):
                wt = work.tile([P, 2 * GD], f32, tag="wtmp", name="wt_qk")
                nc.sync.dma_start(wt[:], wqk[fc * P:(fc + 1) * P, :])
                nc.vector.tensor_copy(wqk_r[:, fc, :], wt[:])
            for fc in range(FC):
                wt = work.tile([P, GD], f32, tag="wtmp", name="wt_v")
                nc.sync.dma_start(wt[:], wv[fc * P:(fc + 1) * P, :])
                nc.vector.tensor_copy(wv_r[:, fc, :], wt[:])
            for cc in range(2):
                wt = work.tile([P, F], f32, tag="wtmp", name="wt_p")
                nc.sync.dma_start(wt[:], wp[cc * P:(cc + 1) * P, :])
                nc.vector.tensor_copy(wp_r[:, cc, :], wt[:])

            # ---- persistent activations ----
            xT = stage.tile([P, FC, S], f32r, tag="xT", name="xT")
            qkT = stage.tile([P, 4, S], f32r, tag="qkT", name="qkT")
            vt = stage.tile([P, HPC, ST, D + 1], f32r, tag="vt", name="vt")
            zTm = stage.tile([P, 2, S], f32r, tag="zTm", name="zTm")
            for h in range(HPC):
                nc.vector.tensor_copy(
                    vt[:, h, :, D:D + 1],
                    ones[:, None, :].to_broadcast((P, ST, 1)),
                )

            def transpose_task(c, tt):
                t = 4 * c + tt
                xt_ = work.tile([P, F], f32, tag="xtile", name="xtile")
                nc.sync.dma_start(xt_[:], x[t * P:(t + 1) * P, :])
                for half in range(2):
                    pp = ps_m.tile([P, CW], f32, tag="mps", name="tps")
                    for q in range(4):
                        fc = half * 4 + q
                        nc.tensor.transpose(
                            pp[:, q * P:(q + 1) * P],
                            xt_[:, fc * P:(fc + 1) * P],
                            ident[:],
                        )
                    nc.vector.tensor_copy(
                        xT[:, half * 4:half * 4 + 4,
                           t * P:(t + 1) * P],
                        pp[:].rearrange("p (f q) -> p f q", f=4),
                    )

            def qkproj_task(c, oc):
                    pp = ps_m.tile([P, CW], f32, tag="mps", name="qkps")
                    for fc in range(FC):
                        nc.tensor.matmul(
                            pp[:],
                            wqk_r[:, fc, oc * P:(oc + 1) * P],
                            xT[:, fc, c * CW:(c + 1) * CW],
                            start=(fc == 0), stop=(fc == FC - 1),
                        )
                    nc.vector.tensor_tensor(
                        qkT[:, oc, c * CW:(c + 1) * CW], pp[:],
                        bqk_sb[:, oc:oc + 1].to_broadcast((P, CW)),
                        mybir.AluOpType.add,
                    )

            def vproj_task(c, tt):
                    t = 4 * c + tt
                    pp = ps_m.tile([P, GD], f32, tag="mps", name="vps")
                    for fc in range(FC):
                        nc.tensor.matmul(
                            pp[:],
                            xT[:, fc, t * P:(t + 1) * P],
                            wv_r[:, fc, :],
                            start=(fc == 0), stop=(fc == FC - 1),
                        )
                    nc.vector.tensor_copy(
                        vt[:, :, t, :D],
                        pp[:].rearrange("p (h d) -> p h d", h=HPC),
                    )

            def av(zp, h, t, ep_ap, col0, ncols, start, stop):
                nc.tensor.matmul(
                    zp[:D + 1, col0:col0 + ncols],
                    vt[:, h, t, :],
                    ep_ap,
                    start=start, stop=stop,
                    skip_group_check=True,
                )

            def scores(sp_ap, h, t, c, q0, qw):
                lo = (h % 2) * D
                nc.tensor.matmul(
                    sp_ap,
                    qkT[lo:lo + D, 2 + h // 2, t * P:(t + 1) * P],
                    qkT[lo:lo + D, h // 2, c * CW + q0:c * CW + q0 + qw],
                    start=True, stop=True,
                    skip_group_check=True,
                )

            def diag_mask(sp_ap):
                nc.vector.tensor_add(sp_ap, sp_ap, mask128[:])

            def attention(c, fillers):
                # insertion points: one after each head's exp emission
                npts = 2 * (2 * c + 2) * 2
                state = {"fi": 0, "pt": 0}

                def fill():
                    state["pt"] += 1
                    left = npts - state["pt"] + 1
                    remaining = len(fillers) - state["fi"]
                    k = (remaining + left - 1) // left if left > 0 else remaining
                    for _ in range(k):
                        fillers[state["fi"]]()
                        state["fi"] += 1

                for hp in range(2):
                    heads = (2 * hp, 2 * hp + 1)
                    zps = [
                        ps_z.tile([P, CW], f32, tag="zps", name=f"zps{i}")
                        for i in range(2)
                    ]
                    # off-diagonal pairs (full width)
                    for pair in range(2 * c):
                        t0, t1 = 2 * pair, 2 * pair + 1
                        for i, h in enumerate(heads):
                            sp = ps_s.tile([P, 2 * CW], f32, tag="sps",
                                           name="sps")
                            scores(sp[:, 0:CW], h, t0, c, 0, CW)
                            scores(sp[:, CW:2 * CW], h, t1, c, 0, CW)
                            ep = eps.tile([P, 2 * CW], f32r, tag="ep",
                                          name="ep")
                            nc.scalar.activation(
                                ep[:], sp[:],
                                mybir.ActivationFunctionType.Exp,
                            )
                            fill()
                            first = (t0 == 0)
                            av(zps[i], h, t0, ep[:, 0:CW], 0, CW,
                               first, False)
                            av(zps[i], h, t1, ep[:, CW:2 * CW], 0, CW,
                               False, False)
                    # diagonal pairs: widths (512, 384) and (256, 128)
                    for dp in range(2):
                        ta, tb = 4 * c + 2 * dp, 4 * c + 2 * dp + 1
                        offa, offb = 2 * dp * P, (2 * dp + 1) * P
                        wa, wb = CW - offa, CW - offb
                        for i, h in enumerate(heads):
                            sp = ps_s.tile([P, 2 * CW], f32, tag="sps",
                                           name="sps")
                            scores(sp[:, 0:wa], h, ta, c, offa, wa)
                            scores(sp[:, wa:wa + wb], h, tb, c, offb, wb)
                            diag_mask(sp[:, 0:P])
                            diag_mask(sp[:, wa:wa + P])
                            ep = eps.tile([P, 2 * CW], f32r, tag="ep",
                                          name="ep")
                            nc.scalar.activation(
                                ep[:, 0:wa + wb], sp[:, 0:wa + wb],
                                mybir.ActivationFunctionType.Exp,
                            )
                            fill()
                            first = (c == 0 and dp == 0)
                            av(zps[i], h, ta, ep[:, 0:wa], offa, wa,
                               first, False)
                            av(zps[i], h, tb, ep[:, wa:wa + wb], offb, wb,
                               False, (dp == 1))
                    # normalize
                    for i, h in enumerate(heads):
                        den = work.tile([1, CW], f32, tag="den", name="den")
                        nc.vector.tensor_copy(den[:], zps[i][D:D + 1, :])
                        rec = work.tile([1, CW], f32, tag="rec", name="rec")
                        nc.vector.reciprocal_approx_fast(rec[:], den[:])
                        recb = work.tile([D, CW], f32, tag="recb",
                                         name="recb")
                        nc.gpsimd.partition_broadcast(recb[:], rec[:])
                        lo = (h % 2) * D
                        nc.vector.tensor_mul(
                            zTm[lo:lo + D, h // 2, c * CW:(c + 1) * CW],
                            zps[i][:D], recb[:],
                        )
                while state["fi"] < len(fillers):
                    fillers[state["fi"]]()
                    state["fi"] += 1

            def outproj_task(c, tt):
                    t = 4 * c + tt
                    osb = work.tile([P, F], f32, tag="osb", name="osb")
                    for n in range(2):
                        pp = ps_m.tile([P, CW], f32, tag="mps", name="ops")
                        for cc in range(2):
                            nc.tensor.matmul(
                                pp[:],
                                zTm[:, cc, t * P:(t + 1) * P],
                                wp_r[:, cc, n * CW:(n + 1) * CW],
                                start=(cc == 0), stop=(cc == 1),
                            )
                        nc.vector.tensor_copy(osb[:, n * CW:(n + 1) * CW],
                                              pp[:])
                    nc.sync.dma_start(out[t * P:(t + 1) * P, :], osb[:])

            def prep_tasks(c):
                tasks = [
                    (lambda tt=tt: transpose_task(c, tt)) for tt in range(4)
                ]
                tasks += [(lambda oc=oc: qkproj_task(c, oc)) for oc in range(4)]
                tasks += [(lambda tt=tt: vproj_task(c, tt)) for tt in range(4)]
                return tasks

            # chunk 0 prep up front, then software-pipeline: during
            # attention(c), weave in outproj(c-1) and all prep for c+1.
            for task in prep_tasks(0):
                task()
            for c in range(SQC):
                fillers = []
                if c > 0:
                    fillers += [
                        (lambda tt=tt, cp=c - 1: outproj_task(cp, tt))
                        for tt in range(4)
                    ]
                if c + 1 < SQC:
                    fillers += prep_tasks(c + 1)
                attention(c, fillers)
            for tt in range(4):
                outproj_task(SQC - 1, tt)
    nc.compile()
    return nc


def make_in_maps(x, w_attn, b_attn, w_proj):
    x = np.ascontiguousarray(np.asarray(x, dtype=np.float32))
    w_attn = np.asarray(w_attn, dtype=np.float32)
    b_attn = np.asarray(b_attn, dtype=np.float32)
    w_proj = np.ascontiguousarray(np.asarray(w_proj, dtype=np.float32))
    scale = np.float32(1.0 / np.sqrt(D))
    in_maps = []
    for core in range(NCORES):
        b, g = divmod(core, 4)
        sl = slice(g * GD, (g + 1) * GD)
        wq = w_attn[:, sl] * scale
        wk = w_attn[:, F + g * GD:F + (g + 1) * GD]
        wqkm = np.ascontiguousarray(
            np.concatenate([wq, wk], axis=1), dtype=np.float32
        )
        wvm = np.ascontiguousarray(
            w_attn[:, 2 * F + g * GD:2 * F + (g + 1) * GD]
        )
        wpg = np.ascontiguousarray(w_proj[sl, :])
        bq = b_attn[sl] * scale
        bk = b_attn[F + g * GD:F + (g + 1) * GD]
        bqkm = np.ascontiguousarray(
            np.concatenate([bq, bk]).reshape(4, P).T, dtype=np.float32
        )
        in_maps.append(
            {"x": np.ascontiguousarray(x[b]), "wqk": wqkm, "wv": wvm,
             "wp": wpg, "bqk": bqkm}
        )
    return in_maps


def assemble(results, b_attn, b_proj, w_proj):
    b_attn = np.asarray(b_attn, dtype=np.float64)
    b_proj = np.asarray(b_proj, dtype=np.float64)
    w_proj = np.asarray(w_proj, dtype=np.float64)
    const = b_attn[2 * F:] @ w_proj + b_proj  # token-independent v-bias term
    full = np.empty((B, S, F), dtype=np.float32)
    for b in range(B):
        acc = results[4 * b]["out"].astype(np.float64)
        for g in range(1, 4):
            acc = acc + results[4 * b + g]["out"]
        full[b] = (acc + const).astype(np.float32)
    return full


def kernel(x, w_attn, b_attn, w_proj, b_proj):
    global _cached_nc
    if _cached_nc is None:
        _cached_nc = build_nc()
    in_maps = make_in_maps(x, w_attn, b_attn, w_proj)
    res = run_bass_kernel_spmd(
        _cached_nc, in_maps, core_ids=list(range(NCORES))
    )
    return assemble(res.results, b_attn, b_proj, w_proj)
